# revision 36
# baseline (speedup 1.0000x reference)
"""Trainium2 Bass kernel for a dense MHA layer (B=2, S=2048, H=1024, 16 heads)
with residual + LayerNorm, tensor-parallel over heads across 8 NeuronCores.

Design (per core c, owning heads 2c and 2c+1):
  QKV: Q/K projections in fp8e4 DoubleRow (two k-tiles per PE pass), bf16
       Q^T/K^T outputs; V in fp8 non-DR (FWL covers the N=128 weight loads),
       stored token-major with a ones column per head so the attention matmul
       also produces softmax denominators.
  attention (software-pipelined at kt granularity): both heads' score matmuls
       (K=64) issue back-to-back into halves of one 2-bank PSUM tile and run
       as concurrent PE row tiles (auto tile_position from base_partition).
       exp of each [128,1024] scores tile is one instruction, split ~11/5
       between ACT (spline Exp -> fp8e5) and DVE (one-op Schraudolph:
       uint8_sat(0.72135*s + 54.057 + 5.7708*mneg) IS the e5m2 bit pattern of
       exp(s/8 + mneg - 1); e5m2's 22-nat range covers the +-9.5-sigma score
       spread, uint8 saturation handles masks, and the global -1 shift
       cancels in the softmax ratio).  att^T = [V|1]^T E in fp8 DoubleRow.
       The previous chunk's att.V matmuls and QKV/phase-4 pieces are
       interleaved into the scores loop so the in-order PE never idles long
       enough for HAM to re-throttle its clock.
  collectives: per-batch AllToAll (head-parallel -> sequence-parallel) with
       interleaved token ownership (core c owns tokens [256c,256c+256) of
       EACH batch): the batch-0 collective + phase 4 overlap batch-1 compute,
       and the batch-1 collective + phase 4 are deferred into the NEXT pass's
       QKV lead-in (steady-state marginal time is what's measured).
  phase 4: denominator reciprocals on 16 partitions, normalizer broadcast via
       a per-j PE outer product (sel 32-scale folds the fp8 ranging), output
       projection in fp8 DoubleRow (Wo pre-scaled by 32, residual by 1024 --
       LayerNorm is scale-invariant), residual add, LayerNorm with a DVE
       Newton rsqrt (avoids ACT table swaps), y in bf16.
"""

import sys

for _p in ("/opt/trn_rl_repo", "/root/.axon_site/_ro/trn_rl_repo"):
    if _p not in sys.path:
        sys.path.append(_p)

import numpy as np
import ml_dtypes

import concourse.bacc as bacc
import concourse.tile as tile
import concourse.mybir as mybir
from concourse.bass_utils import run_bass_kernel_spmd

F32 = mybir.dt.float32
BF16 = mybir.dt.bfloat16
FP8E4 = mybir.dt.float8e4
FP8E5 = mybir.dt.float8e5
U8 = mybir.dt.uint8
AF = mybir.ActivationFunctionType
ALU = mybir.AluOpType
DR = mybir.MatmulPerfMode.DoubleRow

NC = 8          # cores
H = 1024        # model dim
NH = 16         # heads
HD = 64         # head dim
B = 2
S = 2048
T = B * S       # 4096 tokens
TPC = T // NC   # 512 tokens per core (phase 4)
KT = S // 128   # 16 k-tiles per batch
QC = S // 512   # 4 q-chunks of 512 per batch
EPS = 1e-12

# Schraudolph-into-e5m2 constants: bits = EA*s + (EB + EM*mneg).
# e5m2's 22-nat range covers the full softmax range (scores/8 span ~+-9.5);
# e4m3 (12.3 nats) would overflow into NaN/sign-wrapped bits.
EA = 5.7708 / 8.0       # = 0.721350  (score scale 1/8 folded in)
EB = 59.828 - 5.7708    # = 54.0572   (exp bias 15, sigma=.043, C=1 shift)
EM = 5.7708
# exp tiles per q-chunk = 16 (one per kt, both heads); this many go to DVE
DVE_TILES = 5

_RUNNER = None
DEBUG_DUMP = False


def _dve_exp(i):
    """True if exp tile index i (0..15 per q-chunk) goes to DVE."""
    return (i * DVE_TILES) % KT < DVE_TILES


def _build_program(passes=1, single_core=False):
    nc = bacc.Bacc(
        "TRN2",
        target_bir_lowering=False,
        debug=False,
        num_devices=1 if single_core else NC,
    )

    xT = nc.dram_tensor("xT", [H, T], FP8E4, kind="ExternalInput")
    wq = nc.dram_tensor("wq", [H, 128], FP8E4, kind="ExternalInput")
    wk = nc.dram_tensor("wk", [H, 128], FP8E4, kind="ExternalInput")
    wv = nc.dram_tensor("wv", [H, 128], FP8E4, kind="ExternalInput")
    bq = nc.dram_tensor("bq", [128, 1], F32, kind="ExternalInput")
    bk = nc.dram_tensor("bk", [128, 1], F32, kind="ExternalInput")
    bv = nc.dram_tensor("bv", [1, 128], FP8E4, kind="ExternalInput")
    mact = nc.dram_tensor("mact", [128, B * KT], F32, kind="ExternalInput")
    mdve = nc.dram_tensor("mdve", [128, B * KT], F32, kind="ExternalInput")
    wot = nc.dram_tensor("wot", [H, H], FP8E4, kind="ExternalInput")
    resi = nc.dram_tensor("resi", [TPC, H], F32, kind="ExternalInput")
    lnw = nc.dram_tensor("lnw", [128, H], BF16, kind="ExternalInput")
    lnb = nc.dram_tensor("lnb", [128, H], BF16, kind="ExternalInput")
    sel2 = nc.dram_tensor("sel2", [16, 8 * 128], BF16, kind="ExternalInput")
    y = nc.dram_tensor("y", [TPC, H], BF16, kind="ExternalOutput")
    if DEBUG_DUMP:
        dbg_in = nc.dram_tensor("dbg_in", [B, NC, 130, 256], BF16, kind="ExternalOutput")
        dbg_out = nc.dram_tensor("dbg_out", [B, NC, 130, 256], BF16, kind="ExternalOutput")
    else:
        dbg_in = dbg_out = None
    nc._dbg_in, nc._dbg_out = dbg_in, dbg_out

    with tile.TileContext(nc) as tc:
        with (
            tc.tile_pool(name="const", bufs=1) as constp,
            tc.tile_pool(name="pers", bufs=1) as pers,
            tc.tile_pool(name="work", bufs=2) as workp,
            tc.tile_pool(name="ps", bufs=1, space="PSUM") as ps,
            tc.tile_pool(name="dram", bufs=1, space="DRAM") as dram,
        ):
            # ---- constants / weights (fp8, k-pairs along dim 1 for DoubleRow)
            wq_sb = constp.tile([128, 4, 2, 128], FP8E4)
            nc.sync.dma_start(wq_sb[:], wq.ap().rearrange("(d i p) m -> p d i m", p=128, i=2))
            wk_sb = constp.tile([128, 4, 2, 128], FP8E4)
            nc.sync.dma_start(wk_sb[:], wk.ap().rearrange("(d i p) m -> p d i m", p=128, i=2))
            wv_sb = constp.tile([128, 4, 2, 128], FP8E4)
            nc.sync.dma_start(wv_sb[:], wv.ap().rearrange("(d i p) m -> p d i m", p=128, i=2))
            wot_sb = constp.tile([128, 8, H], FP8E4)
            bq_sb = constp.tile([128, 1], F32)
            nc.sync.dma_start(bq_sb[:], bq.ap())
            bk_sb = constp.tile([128, 1], F32)
            nc.sync.dma_start(bk_sb[:], bk.ap())
            bv_sb = constp.tile([1, 128], FP8E4)
            nc.sync.dma_start(bv_sb[:], bv.ap())
            mact_sb = constp.tile([128, B * KT], F32)
            nc.sync.dma_start(mact_sb[:], mact.ap())
            mdve_sb = constp.tile([128, B * KT], F32)
            nc.sync.dma_start(mdve_sb[:], mdve.ap())
            lnw_sb = constp.tile([128, H], BF16)
            nc.sync.dma_start(lnw_sb[:], lnw.ap())
            lnb_sb = constp.tile([128, H], BF16)
            nc.sync.dma_start(lnb_sb[:], lnb.ap())
            ones_sb = constp.tile([1, 128], FP8E4)
            nc.vector.memset(ones_sb[:], 1.0)
            eps_sb = constp.tile([128, 1], F32)
            nc.vector.memset(eps_sb[:], EPS)
            # 2-row selector for the recip broadcast outer product; the 32
            # scale re-ranges fp8 att_sb (matched by wot pre-scale 1/32... see
            # host prep: wot*32, resi*1024).
            # per-j selector: rbp[p,t] = 32*recip16[2j + p//64, t]
            sel2_sb = constp.tile([16, 8, 128], BF16)
            nc.sync.dma_start(sel2_sb[:], sel2.ap())

            # persistent per-pass state (Q/K bf16: scores don't use DoubleRow,
            # so fp8 would buy no PE time and cost accuracy)
            qt_sb = pers.tile([128, T], BF16)   # Q^T (2 heads stacked: 0:64, 64:128)
            kt_sb = pers.tile([128, T], BF16)   # K^T
            # V token-major, per k-tile g: [0:64]=h0, [64]=1, [80:144]=h1, [144]=1
            v_sb = pers.tile([128, 32, 160], FP8E4)
            nc.vector.memset(v_sb[:, :, 64:65], 1.0)
            nc.vector.memset(v_sb[:, :, 144:145], 1.0)

            a2a_in = dram.tile([B, NC, 130, 256], BF16)
            a2a_out = dram.tile([B, NC, 130, 256], BF16)

            xTr = xT.ap().rearrange("(k p) (s t) -> s p k t", p=128, t=512)

            pending = ()
            for _pass in range(passes):
                pending = _emit_body(
                    nc, tc, workp, ps,
                    wq_sb, wk_sb, wv_sb, wot_sb, bq_sb, bk_sb, bv_sb,
                    mact_sb, mdve_sb, lnw_sb, lnb_sb, ones_sb, eps_sb, sel2_sb,
                    qt_sb, kt_sb, v_sb, a2a_in, a2a_out, xTr, resi, y, wot,
                    single_core, pending,
                )
            for ch in pending:
                ch()
            if nc._dbg_in is not None:
                nc.sync.dma_start(nc._dbg_in.ap(), a2a_in[:])
                nc.sync.dma_start(nc._dbg_out.ap(), a2a_out[:])

    nc.compile()
    return nc


class _AttnPipe:
    """Software-pipelined attention emitter.

    Scores+exp for chunk (b,qc) are interleaved at kt granularity with the
    att.V matmuls of the PREVIOUS chunk and with caller-provided extra PE
    work (QKV stripes / phase-4 pieces), so the in-order PE never idles long
    enough for HAM to re-throttle while ACT/DVE chew through the exps.
    """

    def __init__(self, nc, workp, ps, kt_sb, qt_sb, v_sb, mact_sb, mdve_sb,
                 a2a_in):
        self.nc = nc
        self.workp = workp
        self.ps = ps
        self.kt_sb, self.qt_sb, self.v_sb = kt_sb, qt_sb, v_sb
        self.mact_sb, self.mdve_sb = mact_sb, mdve_sb
        self.a2a_in = a2a_in
        self.prev = None  # (b, qc, e_sb) with att.V still to emit
        self.av = None

    def _attv_step(self, step):
        """Emit 2 DoubleRow matmuls of the previous chunk's att.V; steps
        0-3 = head 0 (+avs flush), 4-7 = head 1 (+avs flush)."""
        if self.prev is None:
            return
        nc = self.nc
        pb, pqc, pe = self.prev
        lh, i = step // 4, step % 4
        vc = 80 * lh
        if i == 0:
            self.av = self.ps.tile([65, 512], F32, tag="av", bufs=2)
        for dkt in (2 * i, 2 * i + 1):
            gg = pb * KT + 2 * dkt
            nc.tensor.matmul(
                self.av[:],
                self.v_sb[:, gg : gg + 2, vc : vc + 65],
                pe[:, 2 * dkt : 2 * dkt + 2, lh, :],
                start=(dkt == 0),
                stop=(dkt == KT // 2 - 1),
                perf_mode=DR,
            )
        if i == 3:
            avs = self.workp.tile([65, 512], BF16, tag="avs", bufs=2)
            nc.vector.tensor_copy(avs[:], self.av[:])
            # one DMA for both target cores' att rows, one for the denom rows
            nc.sync.dma_start(
                self.a2a_in[pb, 2 * pqc : 2 * pqc + 2,
                            64 * lh : 64 * lh + 64, :].rearrange(
                                "c p t -> p c t"),
                avs[0:64, :].rearrange("p (c t) -> p c t", c=2),
            )
            nc.sync.dma_start(
                self.a2a_in[pb, 2 * pqc : 2 * pqc + 2,
                            128 + lh : 129 + lh, :].rearrange("c o t -> o c t"),
                avs[64:65, :].rearrange("p (c t) -> p c t", c=2),
            )
            if step == 7:
                self.prev = None

    def drain(self):
        for step in range(8):
            self._attv_step(step)

    def chunk(self, b, qc, extra, every=3):
        """Emit scores+exp for (b,qc), the previous chunk's att.V, and
        callables from `extra` at the slot cadence."""
        nc = self.nc
        qcol = b * S + 512 * qc
        # e tiles: [kt, head, 512] fp8e5 -- per kt, both heads' exps are one
        # contiguous [128,1024] row from a single ACT/DVE instruction
        e_sb = self.workp.tile([128, KT, 2, 512], FP8E5, tag="e", bufs=2)
        for kt in range(KT):
            kcol = b * S + 128 * kt
            g = b * KT + kt
            # both heads into halves of one 2-bank PSUM tile (concurrent row
            # tiles: lhsT/rhs base_partition 0 and 64 -> tile_position auto)
            sp = self.ps.tile([128, 1024], F32, tag="sp", bufs=2)
            for lh in range(2):
                hr = 64 * lh
                nc.tensor.matmul(
                    sp[:, 512 * lh : 512 * (lh + 1)],
                    self.kt_sb[hr : hr + 64, kcol : kcol + 128],
                    self.qt_sb[hr : hr + 64, qcol : qcol + 512],
                    start=True,
                    stop=True,
                )
            if kt % 2 == 0:
                self._attv_step(kt // 2)
            if _dve_exp(kt):
                nc.vector.tensor_scalar(
                    e_sb[:, kt, :, :].bitcast(U8),
                    sp[:],
                    EA,
                    self.mdve_sb[:, g : g + 1],
                    ALU.mult,
                    ALU.add,
                )
            else:
                nc.scalar.activation(
                    e_sb[:, kt, :, :],
                    sp[:],
                    AF.Exp,
                    bias=self.mact_sb[:, g : g + 1],
                    scale=0.125,
                )
            if kt % every == every - 1:
                ch = next(extra, None)
                if ch is not None:
                    ch()
        self.prev = (b, qc, e_sb)


def _phase4_recip(nc, workp, a2a_out, b):
    """Denominator reciprocals for batch b: compute on 16 partitions (DVE's
    iterative divide is per-lane; 2 partitions would be 8x slower), then DMA
    partition-relayout to [2, NC, 256] so each outer-product rhs slice sits
    at base_partition 0 like sel2."""
    sums_sb = workp.tile([16, 256], BF16, tag="sums", bufs=2)
    nc.sync.dma_start(sums_sb[:], a2a_out[b, :, 128:130, :])
    recip16_sb = workp.tile([16, 256], BF16, tag="recip16", bufs=2)
    with nc.allow_low_precision(reason="1/den in bf16; den is O(1e3), 0.4% fine"):
        nc.vector.reciprocal(recip16_sb[:], sums_sb[:])
    return recip16_sb


def _phase4_normalize(nc, workp, ps, sel2_sb, recip_sb, a2a_out, att_sb, b,
                      js):
    for j in js:
        blk = workp.tile([128, 256], BF16, tag="blk", bufs=2)
        nc.sync.dma_start(blk[:], a2a_out[b, j, 0:128, :])
        rbp = ps.tile([128, 256], F32, tag="mm1", bufs=2)
        nc.tensor.matmul(
            rbp[:], sel2_sb[:, j, :], recip_sb[:],
            start=True, stop=True,
        )
        nc.vector.tensor_tensor(att_sb[:, j, :], blk[:], rbp[:], ALU.mult)


def _phase4_proj_ln(nc, workp, ps, wot_sb, lnw_sb, lnb_sb, eps_sb, att_sb,
                    resi, y, b, tts):
    for tt in tts:
        x_sb = workp.tile([128, H], F32, tag="xsb", bufs=2)
        for ft in range(2):
            op = ps.tile([128, 512], F32, tag="mm1", bufs=2)
            for dj in range(4):
                nc.tensor.matmul(
                    op[:],
                    att_sb[:, 2 * dj : 2 * dj + 2, 128 * tt : 128 * (tt + 1)],
                    wot_sb[:, 2 * dj : 2 * dj + 2, 512 * ft : 512 * (ft + 1)],
                    start=(dj == 0),
                    stop=(dj == 3),
                    perf_mode=DR,
                )
            res_t = workp.tile([128, 512], F32, tag="res", bufs=2)
            row0 = 256 * b + 128 * tt
            nc.sync.dma_start(
                res_t[:],
                resi.ap()[row0 : row0 + 128, 512 * ft : 512 * (ft + 1)],
            )
            nc.vector.tensor_tensor(
                x_sb[:, 512 * ft : 512 * (ft + 1)], op[:], res_t[:], ALU.add
            )

        bnst = workp.tile([128, 2, 6], F32, tag="bnst", bufs=2)
        nc.vector.bn_stats(bnst[:, 0, :], x_sb[:, 0:512])
        nc.vector.bn_stats(bnst[:, 1, :], x_sb[:, 512:1024])
        stats = workp.tile([128, 2], F32, tag="stats", bufs=2)
        nc.vector.bn_aggr(stats[:], bnst[:])
        # rstd = 1/sqrt(var) on DVE (quake seed + 2 Newton steps) -- keeps the
        # ACT table set on exp (Sqrt would force two table swaps per pass)
        var = stats[:, 1:2]
        sd_i = workp.tile([128, 1], mybir.dt.int32, tag="sdi", bufs=2)
        nc.vector.tensor_scalar(
            sd_i[:], var.bitcast(mybir.dt.int32), 1, None,
            ALU.arith_shift_right,
        )
        rstd = workp.tile([128, 1], F32, tag="rstd", bufs=2)
        nc.vector.tensor_scalar(
            rstd[:].bitcast(mybir.dt.int32), sd_i[:], -1, 0x5F3759DF,
            ALU.mult, ALU.add,
        )
        hv = workp.tile([128, 1], F32, tag="hv", bufs=2)
        nc.vector.tensor_scalar(hv[:], var, -0.5, None, ALU.mult)
        for _ in range(2):
            yy = workp.tile([128, 1], F32, tag="yy", bufs=2)
            nc.vector.tensor_tensor(yy[:], rstd[:], rstd[:], ALU.mult)
            nc.vector.tensor_tensor(yy[:], yy[:], hv[:], ALU.mult)
            nc.vector.tensor_scalar_add(yy[:], yy[:], 1.5)
            nc.vector.tensor_tensor(rstd[:], rstd[:], yy[:], ALU.mult)
        nmr = workp.tile([128, 1], F32, tag="nmr", bufs=2)
        nc.vector.tensor_scalar(
            nmr[:], stats[:, 0:1], rstd[:], -1.0, ALU.mult, ALU.mult
        )
        xh = workp.tile([128, H], BF16, tag="xh", bufs=2)
        nc.scalar.activation(
            xh[:], x_sb[:], AF.Identity, bias=nmr[:], scale=rstd[:]
        )
        nc.vector.tensor_tensor(xh[:], xh[:], lnw_sb[:], ALU.mult)
        nc.vector.tensor_tensor(xh[:], xh[:], lnb_sb[:], ALU.add)
        row0 = 256 * b + 128 * tt
        nc.sync.dma_start(y.ap()[row0 : row0 + 128, :], xh[:])


def _emit_body(
    nc, tc, workp, ps,
    wq_sb, wk_sb, wv_sb, wot_sb, bq_sb, bk_sb, bv_sb,
    mact_sb, mdve_sb, lnw_sb, lnb_sb, ones_sb, eps_sb, sel2_sb,
    qt_sb, kt_sb, v_sb, a2a_in, a2a_out, xTr, resi, y, wot=None,
    single_core=False, pending=(),
):
    def qkv_stripe(s):
        xs = workp.tile([128, 8, 512], FP8E4, tag="xs", bufs=2)
        nc.sync.dma_start(xs[:], xTr[s])

        qp = ps.tile([128, 512], F32, tag="mm1", bufs=2)
        for d in range(4):
            nc.tensor.matmul(
                qp[:], wq_sb[:, d], xs[:, 2 * d : 2 * d + 2, :],
                start=(d == 0), stop=(d == 3), perf_mode=DR,
            )
        nc.vector.tensor_scalar_add(
            qt_sb[:, 512 * s : 512 * (s + 1)], qp[:], bq_sb[:]
        )

        kp = ps.tile([128, 512], F32, tag="mm1", bufs=2)
        for d in range(4):
            nc.tensor.matmul(
                kp[:], wk_sb[:, d], xs[:, 2 * d : 2 * d + 2, :],
                start=(d == 0), stop=(d == 3), perf_mode=DR,
            )
        nc.vector.tensor_scalar_add(
            kt_sb[:, 512 * s : 512 * (s + 1)], kp[:], bk_sb[:]
        )

        # V: fp8 non-DoubleRow so FWL covers the per-matmul weight loads
        # (these are N=128 passes; DoubleRow would be LDWEIGHTS-bound)
        for tt in range(4):
            vp = ps.tile([128, 128], F32, tag="mm1", bufs=2)
            for k in range(8):
                nc.tensor.matmul(
                    vp[:],
                    xs[:, k, 128 * tt : 128 * (tt + 1)],
                    wv_sb[:, k // 2, k % 2, :],
                    start=(k == 0),
                    stop=False,
                )
            nc.tensor.matmul(vp[:], ones_sb[:], bv_sb[:], start=False, stop=True)
            g = s * 4 + tt
            # one copy, two 64-wide segments (head0 -> col 0, head1 -> col 80)
            nc.vector.tensor_copy(
                v_sb[:, g, :].rearrange("p (u c) -> p u c", u=2)[:, :, 0:64],
                vp[:].rearrange("p (u c) -> p u c", u=2),
            )

    def qkv_chunks(s):
        """Split one QKV stripe into 3 extra-work callables."""
        st = {}

        def c0():
            xs = workp.tile([128, 8, 512], FP8E4, tag="xs", bufs=2)
            nc.sync.dma_start(xs[:], xTr[s])
            st["xs"] = xs
            qp = ps.tile([128, 512], F32, tag="mm1", bufs=2)
            for d in range(4):
                nc.tensor.matmul(
                    qp[:], wq_sb[:, d], xs[:, 2 * d : 2 * d + 2, :],
                    start=(d == 0), stop=(d == 3), perf_mode=DR,
                )
            nc.vector.tensor_scalar_add(
                qt_sb[:, 512 * s : 512 * (s + 1)], qp[:], bq_sb[:]
            )

        def c1():
            xs = st["xs"]
            kp = ps.tile([128, 512], F32, tag="mm1", bufs=2)
            for d in range(4):
                nc.tensor.matmul(
                    kp[:], wk_sb[:, d], xs[:, 2 * d : 2 * d + 2, :],
                    start=(d == 0), stop=(d == 3), perf_mode=DR,
                )
            nc.vector.tensor_scalar_add(
                kt_sb[:, 512 * s : 512 * (s + 1)], kp[:], bk_sb[:]
            )

        def cv(tts):
            def f():
                vstripe(s, st["xs"], tts)
            return f

        return [c0, c1, cv([0, 1]), cv([2, 3])]

    _stripe_cache = {}

    def qkv_chunks_cached(s):
        if s not in _stripe_cache:
            _stripe_cache[s] = qkv_chunks(s)
        return _stripe_cache[s]

    def qkv_chunks2(s):
        return qkv_chunks_cached(s)[2:]

    def vstripe(s, xs, tts):
        for tt in tts:
            vp = ps.tile([128, 128], F32, tag="mm1", bufs=2)
            for k in range(8):
                nc.tensor.matmul(
                    vp[:],
                    xs[:, k, 128 * tt : 128 * (tt + 1)],
                    wv_sb[:, k // 2, k % 2, :],
                    start=(k == 0),
                    stop=False,
                )
            nc.tensor.matmul(vp[:], ones_sb[:], bv_sb[:], start=False, stop=True)
            g = s * 4 + tt
            nc.vector.tensor_copy(
                v_sb[:, g, :].rearrange("p (u c) -> p u c", u=2)[:, :, 0:64],
                vp[:].rearrange("p (u c) -> p u c", u=2),
            )

    def collective(b):
        if single_core:
            nc.sync.dma_start(a2a_out[b], a2a_in[b])
        else:
            nc.gpsimd.collective_compute(
                "AllToAll",
                ALU.bypass,
                replica_groups=[list(range(NC))],
                ins=[a2a_in[b].opt()],
                outs=[a2a_out[b].opt()],
            )

    pipe = _AttnPipe(nc, workp, ps, kt_sb, qt_sb, v_sb, mact_sb, mdve_sb,
                     a2a_in)

    # ---- batch 0 QKV up front (previous pass's phase-4 pieces
    # interleaved -- they wait on the previous collective, hiding its
    # latency under this pass's QKV), then attention(0) with batch-1 QKV
    # chunks feeding the PE during exp waits
    qkv_stripe(0)
    # per stripe: q, k, v01, v23 in order -- scores kt 4j..4j+3 only need
    # stripe j's K (emitted 3 slots earlier), and only 2 xs tiles are ever
    # live so the xs pool (bufs=2) can't deadlock the in-order PE queue
    lead = [c for s in range(1, 4) for c in qkv_chunks_cached(s)]
    # extra-work order: batch-1 QKV chunks first; the previous pass's
    # deferred phase-4 pieces (which wait on its trailing collective) are
    # spliced in only from the second half on, so they can't stall the
    # in-order PE queue while that collective is still in flight.
    qkvc = [c for s in range(4, 8) for c in qkv_chunks(s)]
    pend = list(pending)
    order = qkvc[:10]
    rest = qkvc[10:]
    for i in range(max(len(pend), len(rest))):
        if i < len(pend):
            order.append(pend[i])
        if i < len(rest):
            order.append(rest[i])
    pipe.chunk(0, 0, iter(lead), every=1)
    extra0 = iter(order)
    for qc in range(1, QC):
        pipe.chunk(0, qc, extra0)
    for ch in extra0:
        ch()

    # ---- attention(1); att.V(0,3) drains inside chunk (1,0), then cc(0)
    att0_sb = workp.tile([128, 8, 256], FP8E4, tag="att", bufs=2)
    p4 = []
    p4.append(lambda: _phase4_normalize(nc, workp, ps, sel2_sb, st4["r"],
                                        a2a_out, att0_sb, 0, range(0, 3)))
    p4.append(lambda: _phase4_normalize(nc, workp, ps, sel2_sb, st4["r"],
                                        a2a_out, att0_sb, 0, range(3, 6)))
    p4.append(lambda: _phase4_normalize(nc, workp, ps, sel2_sb, st4["r"],
                                        a2a_out, att0_sb, 0, range(6, 8)))
    p4.append(lambda: _phase4_proj_ln(nc, workp, ps, wot_sb, lnw_sb, lnb_sb,
                                      eps_sb, att0_sb, resi, y, 0, [0]))
    p4.append(lambda: _phase4_proj_ln(nc, workp, ps, wot_sb, lnw_sb, lnb_sb,
                                      eps_sb, att0_sb, resi, y, 0, [1]))
    st4 = {}
    empty = iter([])
    pipe.chunk(1, 0, empty)
    collective(0)
    # wot load overlaps the collective
    nc.sync.dma_start(wot_sb[:], wot.ap().rearrange("(j p) f -> p j f", p=128))
    st4["r"] = _phase4_recip(nc, workp, a2a_out, 0)
    extra1 = iter(p4)
    for qc in range(1, QC):
        pipe.chunk(1, qc, extra1)
    pipe.drain()
    for ch in extra1:
        ch()
    collective(1)

    # ---- phase 4 for batch 1: deferred into the next pass's QKV lead-in
    stb = {}

    def d0():
        stb["att"] = workp.tile([128, 8, 256], FP8E4, tag="att", bufs=2,
                                name="att1_sb")
        stb["r"] = _phase4_recip(nc, workp, a2a_out, 1)

    def dnorm(js):
        def f():
            _phase4_normalize(nc, workp, ps, sel2_sb, stb["r"], a2a_out,
                              stb["att"], 1, js)
        return f

    def dproj(tts):
        def f():
            _phase4_proj_ln(nc, workp, ps, wot_sb, lnw_sb, lnb_sb, eps_sb,
                            stb["att"], resi, y, 1, tts)
        return f

    return [d0, dnorm(range(0, 4)), dnorm(range(4, 8)), dproj([0]), dproj([1])]


class _Runner:
    """Compiles the Bass program once and keeps a reusable sharded jit."""

    def __init__(self, build_fn=None):
        self.nc = (build_fn or _build_program)()
        self._sharded = None
        self._meta = None

    def _make_sharded(self):
        import jax
        from jax.sharding import Mesh, PartitionSpec
        from jax.experimental.shard_map import shard_map
        from concourse.bass2jax import (
            _bass_exec_p,
            install_neuronx_cc_hook,
            partition_id_tensor,
        )

        install_neuronx_cc_hook()
        nc = self.nc
        partition_name = (
            nc.partition_id_tensor.name if nc.partition_id_tensor else None
        )

        in_names, out_names, out_avals, zero_outs = [], [], [], []
        for alloc in nc.m.functions[0].allocations:
            if not isinstance(alloc, mybir.MemoryLocationSet):
                continue
            name = alloc.memorylocations[0].name
            if alloc.kind == "ExternalInput":
                if name != partition_name:
                    in_names.append(name)
            elif alloc.kind == "ExternalOutput":
                shape = tuple(alloc.tensor_shape)
                dtype = mybir.dt.np(alloc.dtype)
                out_names.append(name)
                out_avals.append(jax.core.ShapedArray(shape, dtype))
                zero_outs.append(np.zeros(shape, dtype))
        n_params = len(in_names)
        all_names = list(in_names) + list(out_names)
        if partition_name is not None:
            all_names.append(partition_name)

        def _body(*args):
            operands = list(args)
            if partition_name is not None:
                operands.append(partition_id_tensor())
            outs = _bass_exec_p.bind(
                *operands,
                out_avals=tuple(out_avals),
                in_names=tuple(all_names),
                out_names=tuple(out_names),
                lowering_input_output_aliases=(),
                sim_require_finite=True,
                sim_require_nnan=True,
                nc=nc,
            )
            return tuple(outs)

        devices = jax.devices()[:NC]
        mesh = Mesh(np.asarray(devices), ("core",))
        self._mesh = mesh
        n_outs = len(out_names)
        in_specs = (PartitionSpec("core"),) * (n_params + n_outs)
        out_specs = (PartitionSpec("core"),) * n_outs
        donate = tuple(range(n_params, n_params + n_outs))
        sharded = jax.jit(
            shard_map(
                _body, mesh=mesh, in_specs=in_specs, out_specs=out_specs, check_rep=False
            ),
            donate_argnums=donate,
            keep_unused=True,
        )
        self._meta = (in_names, out_names, out_avals, zero_outs)
        self._sharded = sharded

    def stage_inputs(self, in_maps):
        """device_put the concatenated inputs once; returns (ins_dev, zeros_dev)."""
        import jax
        from jax.sharding import NamedSharding, PartitionSpec

        if self._sharded is None:
            self._make_sharded()
        in_names, out_names, out_avals, zero_outs = self._meta
        sh = NamedSharding(self._mesh, PartitionSpec("core"))
        concat_in = [
            np.concatenate([np.asarray(m[name]) for m in in_maps], axis=0)
            for name in in_names
        ]
        concat_zeros = [
            np.zeros((NC * z.shape[0], *z.shape[1:]), z.dtype) for z in zero_outs
        ]
        ins_dev = [jax.device_put(a, sh) for a in concat_in]
        zeros_dev = [jax.device_put(a, sh) for a in concat_zeros]
        return ins_dev, zeros_dev

    def bench(self, in_maps, iters=20):
        """Steady-state seconds/call with device-resident inputs."""
        import jax
        import time

        ins_dev, zeros_dev = self.stage_inputs(in_maps)
        outs = self._sharded(*ins_dev, *zeros_dev)
        jax.block_until_ready(outs)
        t0 = time.time()
        for _ in range(iters):
            outs = self._sharded(*ins_dev, *outs)
        jax.block_until_ready(outs)
        return (time.time() - t0) / iters

    def run(self, in_maps):
        if self._sharded is None:
            self._make_sharded()
        in_names, out_names, out_avals, zero_outs = self._meta
        concat_in = [
            np.concatenate([np.asarray(m[name]) for m in in_maps], axis=0)
            for name in in_names
        ]
        concat_zeros = [
            np.zeros((NC * z.shape[0], *z.shape[1:]), z.dtype) for z in zero_outs
        ]
        out_arrs = self._sharded(*concat_in, *concat_zeros)
        return [
            {
                name: np.asarray(out_arrs[i]).reshape(NC, *out_avals[i].shape)[c]
                for i, name in enumerate(out_names)
            }
            for c in range(NC)
        ]


def _get_runner():
    global _RUNNER
    if _RUNNER is None:
        _RUNNER = _Runner()
    return _RUNNER


def _prep_in_maps(pre_out, att_mask, Wq, bq, Wk, bk, Wv, bv, Wo, bo, ln_w, ln_b):
    f32 = np.float32
    bf16 = ml_dtypes.bfloat16
    f8 = ml_dtypes.float8_e4m3
    x = np.asarray(pre_out, f32).reshape(T, H)
    xT = np.ascontiguousarray(x.T).astype(f8)

    m = (1.0 - np.asarray(att_mask, f32).reshape(B, S)) * -10000.0
    # column (b*KT + kt) holds mask for k-tokens [kt*128, (kt+1)*128) of batch b
    mneg = np.ascontiguousarray(
        m.reshape(B, KT, 128).transpose(2, 0, 1).reshape(128, B * KT)
    )
    mact = mneg - 1.0
    mdve = EB + EM * mneg

    wot = np.ascontiguousarray(np.asarray(Wo, f32).T * 32.0).astype(f8)
    # token-interleaved residual: rows [0:256) = batch0 tokens [256c,256c+256),
    # rows [256:512) = batch1 same; pre-scaled by 1024 (LN is scale-invariant)
    res_full = (x + np.asarray(bo, f32)[None, :]) * 1024.0
    lnw_b = np.ascontiguousarray(
        np.broadcast_to(np.asarray(ln_w, f32), (128, H))
    ).astype(bf16)
    lnb_b = np.ascontiguousarray(
        np.broadcast_to(np.asarray(ln_b, f32), (128, H))
    ).astype(bf16)

    Wq_, Wk_, Wv_ = (np.asarray(w, f32) for w in (Wq, Wk, Wv))
    bq_, bk_, bv_ = (np.asarray(v, f32) for v in (bq, bk, bv))

    sel2 = np.zeros((16, 8, 128), np.float32)
    for j in range(8):
        sel2[2 * j, j, 0:64] = 32.0
        sel2[2 * j + 1, j, 64:128] = 32.0
    sel2 = sel2.reshape(16, 1024).astype(bf16)

    in_maps = []
    for c in range(NC):
        fs = slice(128 * c, 128 * (c + 1))
        res_c = np.concatenate(
            [
                res_full[0 * S + 256 * c : 0 * S + 256 * (c + 1)],
                res_full[1 * S + 256 * c : 1 * S + 256 * (c + 1)],
            ],
            axis=0,
        )
        in_maps.append(
            {
                "xT": xT,
                "wq": np.ascontiguousarray(Wq_[fs].T).astype(f8),
                "wk": np.ascontiguousarray(Wk_[fs].T).astype(f8),
                "wv": np.ascontiguousarray(Wv_[fs].T).astype(f8),
                "bq": np.ascontiguousarray(bq_[fs].reshape(128, 1)),
                "bk": np.ascontiguousarray(bk_[fs].reshape(128, 1)),
                "bv": np.ascontiguousarray(bv_[fs].reshape(1, 128)).astype(f8),
                "mact": mact,
                "mdve": mdve,
                "wot": wot,
                "resi": np.ascontiguousarray(res_c),
                "lnw": lnw_b,
                "lnb": lnb_b,
                "sel2": sel2,
            }
        )
    return in_maps


def kernel(**inputs):
    runner = _get_runner()
    in_maps = _prep_in_maps(**inputs)
    results = runner.run(in_maps)
    # un-interleave: core c rows [0:256) = batch0 tokens [256c, 256c+256),
    # rows [256:512) = batch1 same
    y = np.stack([results[c]["y"] for c in range(NC)], axis=0)  # [NC, 512, H]
    y = y.reshape(NC, B, 256, H).transpose(1, 0, 2, 3).reshape(B, S, H)
    return y.astype(np.float32)
